# revision 18
# baseline (speedup 1.0000x reference)
"""Causal multi-head attention (B=4, S=2048, D=1024, H=16) on 8 TRN2 NeuronCores.

Sharding: DP=4 over batch x TP=2 over heads (8 heads per core). Each core:
  - receives transposed activations xT = x[b].T (host-prepared, bf16),
    column shards of Wq/Wk/Wv (512 cols = 8 heads) and the row shard of Wo.
  - computes V (natural layout, with a ones-column per head that yields the
    softmax denominators inside the PV matmul), then per head-pair p:
    KT[p]/QT[p] projections -> scoresT = K_h Q_h^T (2-head row-packed
    matmuls, causal tile skipping) -> probsT = exp(scoresT/8) * causal mask
    -> PV -> numerator^T + denominator -> batched-reciprocal normalization
    -> normalized A^T pair-chunk shipped to DRAM,
    so the ACT-bound attention pipeline overlaps the projection matmuls.
  - finally the partial output A^T.T @ Wo_shard in [seq, D] layout.
  - host sums the two TP partials per batch and adds bo.

All matmul operands are bf16 (1 cycle/column on the PE, half the DMA bytes,
2x DVE modes); accumulation and softmax normalization stay fp32 in PSUM.
All x-activation pools stay live together so no DMA ever waits on compute
for SBUF space (head-of-line blocking on the DMA queue); xv streams on the
gpsimd queue, everything else on sync.
"""

import sys

sys.path.insert(0, "/opt/trn_rl_repo")

import numpy as np

B = 4
S = 2048
D = 1024
H = 16
HD = 64
TP = 2
DH = D // TP          # 512 head-dims per core (8 heads)
NHL = DH // HD        # 8 local heads
DCH = 4               # dchunks of 128 within DH
NKT = S // 128        # 16 key tiles
NQT = S // 512        # 4 query tiles
KCH = D // 128        # 8 contraction tiles for projections
GRP = 2               # score k-tiles grouped per exp op

_compiled = None


def _build():
    import concourse.bacc as bacc
    import concourse.mybir as mybir
    import concourse.tile as tile

    F32 = mybir.dt.float32
    BF16 = mybir.dt.bfloat16
    EXP = mybir.ActivationFunctionType.Exp

    nc = bacc.Bacc("TRN2", target_bir_lowering=False, debug=False)

    xq = nc.dram_tensor("xq", [D, S], BF16, kind="ExternalInput")
    xk = nc.dram_tensor("xk", [D, S], BF16, kind="ExternalInput")
    xv = nc.dram_tensor("xv", [D, S], BF16, kind="ExternalInput")
    wq = nc.dram_tensor("wq", [D, DH], BF16, kind="ExternalInput")
    wk = nc.dram_tensor("wk", [D, DH], BF16, kind="ExternalInput")
    wv = nc.dram_tensor("wv", [D, DH], BF16, kind="ExternalInput")
    wo = nc.dram_tensor("wo", [DH, D], BF16, kind="ExternalInput")
    bq_c = nc.dram_tensor("bq_c", [128, DCH], F32, kind="ExternalInput")
    bk_c = nc.dram_tensor("bk_c", [128, DCH], F32, kind="ExternalInput")
    bv_b = nc.dram_tensor("bv_b", [128, DH], F32, kind="ExternalInput")
    out = nc.dram_tensor("out", [S, D], F32, kind="ExternalOutput")
    dden = nc.dram_tensor("dden", [NHL, S], F32)    # denominators bounce
    rden = nc.dram_tensor("rden", [NHL, S], F32)    # reciprocals bounce

    with tile.TileContext(nc) as tc:
        with (
            tc.tile_pool(name="qt", bufs=1) as qt_pool,
            tc.tile_pool(name="kt", bufs=1) as kt_pool,
            tc.tile_pool(name="vn", bufs=1) as vn_pool,
            tc.tile_pool(name="cst", bufs=1) as cst,
        ):
            QT = [qt_pool.tile([128, S], BF16, tag=f"qt{d}", name=f"QT{d}")
                  for d in range(DCH)]
            KT = [kt_pool.tile([128, S], BF16, tag=f"kt{d}", name=f"KT{d}")
                  for d in range(DCH)]
            # V natural [seq, 8*(64+1)]: head h cols 65h..65h+63, ones at 65h+64
            VN = [vn_pool.tile([128, NHL * (HD + 1)], BF16, tag=f"vn{i}",
                               name=f"VN{i}")
                  for i in range(NKT)]

            bqs = cst.tile([128, DCH], F32, tag="bqs", name="bqs")
            bks = cst.tile([128, DCH], F32, tag="bks", name="bks")
            bvb = cst.tile([128, DH], F32, tag="bvb", name="bvb")
            nc.sync.dma_start(out=bqs[:, :], in_=bq_c[:, :])
            nc.sync.dma_start(out=bks[:, :], in_=bk_c[:, :])
            nc.sync.dma_start(out=bvb[:, :], in_=bv_b[:, :])

            # causal mask base [128, 896]: mask[x, c] = 1.0 iff c - x >= 384.
            # crossing k-tile i (0..3) of a 512-q tile uses slice
            # mask[:, 384-128i : 896-128i]  ->  valid iff y >= x + 128 i.
            mask = cst.tile([128, 896], BF16, tag="mask", name="mask")
            nc.gpsimd.memset(mask[:, :], 1.0)
            nc.gpsimd.affine_select(
                out=mask[:, :],
                in_=mask[:, :],
                compare_op=mybir.AluOpType.is_ge,
                fill=0.0,
                base=-384,
                pattern=[[1, 896]],
                channel_multiplier=-1,
            )

            ones = cst.tile([128, NHL], F32, tag="ones", name="ones")
            nc.vector.memset(ones[:, :], 1.0)
            for v in VN:
                nc.vector.tensor_copy(v[:, HD::HD + 1], ones[:, :])

            atp_ctx = tc.tile_pool(name="atp", bufs=4)
            atp_pool = atp_ctx.__enter__()
            with (
                tc.tile_pool(name="xv", bufs=1) as xv_pool,
                tc.tile_pool(name="xkq", bufs=1) as xkq_pool,
                tc.tile_pool(name="wp", bufs=1) as wp_pool,
                tc.tile_pool(name="pr", bufs=4) as pr_pool,
                tc.tile_pool(name="nrm", bufs=2) as nrm_pool,
                tc.tile_pool(name="psA", bufs=2, space="PSUM") as psA,
                tc.tile_pool(name="psS", bufs=2, space="PSUM") as psS,
                tc.tile_pool(name="psV", bufs=1, space="PSUM") as psV,
            ):
                wvt = [wp_pool.tile([128, DH], BF16, tag=f"wv{ki}",
                                    name=f"wv{ki}")
                       for ki in range(KCH)]
                wkt = [wp_pool.tile([128, DH], BF16, tag=f"wk{ki}",
                                    name=f"wk{ki}")
                       for ki in range(KCH)]
                wqt = [wp_pool.tile([128, DH], BF16, tag=f"wq{ki}",
                                    name=f"wq{ki}")
                       for ki in range(KCH)]
                xkt = [xkq_pool.tile([128, S], BF16, tag=f"xk{ki}",
                                     name=f"xk{ki}")
                       for ki in range(KCH)]
                xqt = [xkq_pool.tile([128, S], BF16, tag=f"xq{ki}",
                                     name=f"xq{ki}")
                       for ki in range(KCH)]
                for ki in range(KCH):
                    nc.sync.dma_start(out=wvt[ki][:, :],
                                      in_=wv[128 * ki:128 * (ki + 1), :])
                for ki in range(KCH):
                    nc.sync.dma_start(out=wkt[ki][:, :],
                                      in_=wk[128 * ki:128 * (ki + 1), :])
                    nc.sync.dma_start(out=wqt[ki][:, :],
                                      in_=wq[128 * ki:128 * (ki + 1), :])
                for ki in range(KCH):
                    nc.sync.dma_start(out=xkt[ki][:, :],
                                      in_=xk[128 * ki:128 * (ki + 1), :])
                    nc.sync.dma_start(out=xqt[ki][:, :],
                                      in_=xq[128 * ki:128 * (ki + 1), :])

                # ---- V projection (xv streamed in halves on gpsimd queue) --
                for half in range(2):
                    s0 = 1024 * half
                    xvt = [xv_pool.tile([128, 1024], BF16, tag=f"xv{ki}",
                                        name=f"xv{ki}_")
                           for ki in range(KCH)]
                    for ki in range(KCH):
                        nc.gpsimd.dma_start(
                            out=xvt[ki][:, :],
                            in_=xv[128 * ki:128 * (ki + 1), s0:s0 + 1024])
                    for st in range(8):
                        ps = psA.tile([128, DH], F32, tag="psA", name="psAv_")
                        for ki in range(KCH):
                            nc.tensor.matmul(
                                ps[:, :],
                                xvt[ki][:, 128 * st:128 * (st + 1)],
                                wvt[ki][:, :],
                                start=(ki == 0),
                                stop=(ki == KCH - 1),
                            )
                        vdst = VN[8 * half + st][:, :].rearrange(
                            "p (h c) -> p h c", c=HD + 1)[:, :, :HD]
                        nc.vector.tensor_add(
                            vdst,
                            ps[:, :].rearrange("p (h c) -> p h c", c=HD),
                            bvb[:, :].rearrange("p (h c) -> p h c", c=HD),
                        )

                # ---- K/Q projections (all pairs) ----
                for p in range(DCH):
                    for wt, xt, dest, bias in ((wkt, xkt, KT, bks),
                                               (wqt, xqt, QT, bqs)):
                        for sc in range(NQT):
                            ps = psA.tile([128, 512], F32, tag="psA",
                                          name="psA_")
                            for ki in range(KCH):
                                nc.tensor.matmul(
                                    ps[:, :],
                                    wt[ki][:, 128 * p:128 * (p + 1)],
                                    xt[ki][:, 512 * sc:512 * (sc + 1)],
                                    start=(ki == 0),
                                    stop=(ki == KCH - 1),
                                )
                            nc.vector.tensor_scalar_add(
                                dest[p][:, 512 * sc:512 * (sc + 1)],
                                ps[:, :],
                                bias[:, p:p + 1],
                            )

                # ---- attention per head pair ----
                atp_tiles = []
                for p in range(DCH):
                    atp = atp_pool.tile([128, S], BF16, tag="atp",
                                        name="atp_")
                    atp_tiles.append(atp)
                    for j in range(NQT):
                        q0 = 512 * j
                        nk = 4 * (j + 1)  # valid k-tiles (causal)
                        pv = [psV.tile([HD + 1, 512], F32, tag=f"pv{h}",
                                       name=f"pv{h}_")
                              for h in range(2)]
                        for g in range(0, nk, GRP):
                            pss = [psS.tile([128, 512 * GRP], F32, tag="psS",
                                            name="psS_")
                                   for _ in range(2)]
                            for m in range(GRP):
                                k = g + m
                                for h in range(2):
                                    r0 = 64 * h
                                    nc.tensor.matmul(
                                        pss[h][:, 512 * m:512 * (m + 1)],
                                        KT[p][r0:r0 + 64,
                                              128 * k:128 * (k + 1)],
                                        QT[p][r0:r0 + 64, q0:q0 + 512],
                                        start=True,
                                        stop=True,
                                    )
                            prt = [pr_pool.tile([128, 512 * GRP], BF16,
                                                tag="pr", name="pr_")
                                   for _ in range(2)]
                            for h in range(2):
                                nc.scalar.activation(
                                    prt[h][:, :], pss[h][:, :], EXP,
                                    scale=0.125)
                            # causal mask on diagonal-crossing k-tiles
                            for h in range(2):
                                for m in range(GRP):
                                    i = g + m - 4 * j
                                    if 0 <= i <= 3:
                                        msl = mask[:, 384 - 128 * i:
                                                   896 - 128 * i]
                                        nc.vector.tensor_mul(
                                            prt[h][:, 512 * m:512 * (m + 1)],
                                            prt[h][:, 512 * m:512 * (m + 1)],
                                            msl,
                                        )
                            for m in range(GRP):
                                k = g + m
                                for h in range(2):
                                    hl = 2 * p + h
                                    nc.tensor.matmul(
                                        pv[h][:, :],
                                        VN[k][:, 65 * hl:65 * hl + 65],
                                        prt[h][:, 512 * m:512 * (m + 1)],
                                        start=(k == 0),
                                        stop=(k == nk - 1),
                                    )
                        # fast drain: unnormalized numerator -> atp,
                        # denominator row -> DRAM bounce (frees the pv slot)
                        for h in range(2):
                            hl = 2 * p + h
                            nc.vector.tensor_copy(
                                atp[64 * h:64 * h + 64, q0:q0 + 512],
                                pv[h][:HD, :],
                            )
                            drow = nrm_pool.tile([1, 512], F32, tag="drow",
                                                 name="drow_")
                            nc.vector.tensor_copy(drow[:, :],
                                                  pv[h][HD:HD + 1, :])
                            nc.sync.dma_start(
                                out=dden[hl:hl + 1, q0:q0 + 512],
                                in_=drow[:, :],
                            )

                    # per-pair batched normalization, then ship A^T chunk
                    dd = nrm_pool.tile([128, 32], F32, tag="dd", name="dd_")
                    nc.sync.dma_start(
                        out=dd[:, :],
                        in_=dden[2 * p:2 * p + 2, :].rearrange(
                            "h (a f) -> (h a) f", f=32),
                    )
                    rc = nrm_pool.tile([128, 32], F32, tag="rc", name="rc_")
                    nc.vector.reciprocal(rc[:, :], dd[:, :])
                    nc.sync.dma_start(
                        out=rden[2 * p:2 * p + 2, :].rearrange(
                            "h (a f) -> (h a) f", f=32),
                        in_=rc[:, :],
                    )
                    for j in range(NQT):
                        q0 = 512 * j
                        bct = nrm_pool.tile([128, 512], F32, tag="bct",
                                            name="bct_")
                        for h in range(2):
                            nc.sync.dma_start(
                                out=bct[64 * h:64 * h + 64, :],
                                in_=rden[2 * p + h:2 * p + h + 1, q0:q0 + 512]
                                .partition_broadcast(64),
                            )
                        nc.vector.tensor_mul(
                            atp[:, q0:q0 + 512],
                            atp[:, q0:q0 + 512],
                            bct[:, :],
                        )

            # ---------------- Output projection ----------------
            with (
                tc.tile_pool(name="wo", bufs=1) as wo_pool,
                tc.tile_pool(name="ob", bufs=4) as ob_pool,
                tc.tile_pool(name="psO", bufs=4, space="PSUM") as psO,
            ):
                wot = [wo_pool.tile([128, D], BF16, tag=f"wo{c}",
                                    name=f"wot{c}")
                       for c in range(DCH)]
                for c in range(DCH):
                    nc.sync.dma_start(
                        out=wot[c][:, :], in_=wo[128 * c:128 * (c + 1), :])
                for qt in range(NKT):  # 16 q tiles of 128
                    q0 = 128 * qt
                    for n in range(2):
                        ps = psO.tile([128, 512], F32, tag="psO", name="psO_")
                        for c in range(DCH):
                            nc.tensor.matmul(
                                ps[:, :],
                                atp_tiles[c][:, q0:q0 + 128],
                                wot[c][:, 512 * n:512 * (n + 1)],
                                start=(c == 0),
                                stop=(c == DCH - 1),
                            )
                        ot = ob_pool.tile([128, 512], F32, tag="ob",
                                          name="ob_")
                        nc.vector.tensor_copy(ot[:, :], ps[:, :])
                        nc.sync.dma_start(
                            out=out[q0:q0 + 128, 512 * n:512 * (n + 1)],
                            in_=ot[:, :])
            atp_ctx.__exit__(None, None, None)

    nc.compile()
    return nc


def kernel(query, key, value, Wq, bq, Wk, bk, Wv, bv, Wo, bo, **trace_kwargs):
    from concourse.bass_utils import run_bass_kernel_spmd

    global _compiled
    if _compiled is None:
        _compiled = _build()
    nc = _compiled

    import ml_dtypes

    BF = ml_dtypes.bfloat16
    query = np.asarray(query, np.float32)
    key = np.asarray(key, np.float32)
    value = np.asarray(value, np.float32)
    Wq, Wk, Wv, Wo = (np.asarray(w, np.float32) for w in (Wq, Wk, Wv, Wo))
    bq, bk, bv, bo = (np.asarray(b_, np.float32) for b_ in (bq, bk, bv, bo))

    xqT = [np.ascontiguousarray(query[b].T).astype(BF) for b in range(B)]
    xkT = [np.ascontiguousarray(key[b].T).astype(BF) for b in range(B)]
    xvT = [np.ascontiguousarray(value[b].T).astype(BF) for b in range(B)]
    shard = []
    for t in range(TP):
        cs = slice(DH * t, DH * (t + 1))
        shard.append({
            "wq": np.ascontiguousarray(Wq[:, cs]).astype(BF),
            "wk": np.ascontiguousarray(Wk[:, cs]).astype(BF),
            "wv": np.ascontiguousarray(Wv[:, cs]).astype(BF),
            "wo": np.ascontiguousarray(Wo[cs, :]).astype(BF),
            "bq_c": np.ascontiguousarray(bq[cs].reshape(DCH, 128).T),
            "bk_c": np.ascontiguousarray(bk[cs].reshape(DCH, 128).T),
            "bv_b": np.ascontiguousarray(
                np.broadcast_to(bv[cs], (128, DH))),
        })

    in_maps = []
    for c in range(8):
        b, t = c // TP, c % TP
        m = {"xq": xqT[b], "xk": xkT[b], "xv": xvT[b]}
        m.update(shard[t])
        in_maps.append(m)

    res = run_bass_kernel_spmd(nc, in_maps, core_ids=list(range(8)),
                               **trace_kwargs)
    outp = np.empty((B, S, D), np.float32)
    for b in range(B):
        outp[b] = res.results[TP * b]["out"] + res.results[TP * b + 1]["out"] + bo
    if trace_kwargs:
        kernel.last_results = res
    return outp


# revision 19
# speedup vs baseline: 1.0577x; 1.0577x over previous
"""Causal multi-head attention (B=4, S=2048, D=1024, H=16) on 8 TRN2 NeuronCores.

Sharding: DP=4 over batch x TP=2 over heads (8 heads per core). Each core:
  - receives transposed activations xT = x[b].T (host-prepared, bf16),
    column shards of Wq/Wk/Wv (512 cols = 8 heads) and the row shard of Wo.
  - computes V (natural layout, with a ones-column per head that yields the
    softmax denominators inside the PV matmul), then per head-pair p:
    KT[p]/QT[p] projections -> scoresT = K_h Q_h^T (2-head row-packed
    matmuls, causal tile skipping) -> probsT = exp(scoresT/8) * causal mask
    -> PV -> numerator^T + denominator -> batched-reciprocal normalization
    (A^T pair-chunks stay resident in SBUF for the output projection),
    so the ACT-bound attention pipeline overlaps the projection matmuls.
  - finally the partial output A^T.T @ Wo_shard in [seq, D] layout.
  - host sums the two TP partials per batch and adds bo.

All matmul operands are bf16 (1 cycle/column on the PE, half the DMA bytes,
2x DVE modes); accumulation and softmax normalization stay fp32 in PSUM.
All x-activation pools stay live together so no DMA ever waits on compute
for SBUF space (head-of-line blocking on the DMA queue); xv streams on the
gpsimd queue, everything else on sync.
"""

import sys

sys.path.insert(0, "/opt/trn_rl_repo")

import numpy as np

B = 4
S = 2048
D = 1024
H = 16
HD = 64
TP = 2
DH = D // TP          # 512 head-dims per core (8 heads)
NHL = DH // HD        # 8 local heads
DCH = 4               # dchunks of 128 within DH
NKT = S // 128        # 16 key tiles
NQT = S // 512        # 4 query tiles
KCH = D // 128        # 8 contraction tiles for projections
GRP = 2               # score k-tiles grouped per exp op

_compiled = None


def _build():
    import concourse.bacc as bacc
    import concourse.mybir as mybir
    import concourse.tile as tile

    F32 = mybir.dt.float32
    BF16 = mybir.dt.bfloat16
    EXP = mybir.ActivationFunctionType.Exp

    nc = bacc.Bacc("TRN2", target_bir_lowering=False, debug=False)

    xq = nc.dram_tensor("xq", [D, S], BF16, kind="ExternalInput")
    xk = nc.dram_tensor("xk", [D, S], BF16, kind="ExternalInput")
    xv = nc.dram_tensor("xv", [D, S], BF16, kind="ExternalInput")
    wq = nc.dram_tensor("wq", [D, DH], BF16, kind="ExternalInput")
    wk = nc.dram_tensor("wk", [D, DH], BF16, kind="ExternalInput")
    wv = nc.dram_tensor("wv", [D, DH], BF16, kind="ExternalInput")
    wo = nc.dram_tensor("wo", [DH, D], BF16, kind="ExternalInput")
    bq_c = nc.dram_tensor("bq_c", [128, DCH], F32, kind="ExternalInput")
    bk_c = nc.dram_tensor("bk_c", [128, DCH], F32, kind="ExternalInput")
    bv_b = nc.dram_tensor("bv_b", [128, DH], F32, kind="ExternalInput")
    out = nc.dram_tensor("out", [S, D], F32, kind="ExternalOutput")
    dden = nc.dram_tensor("dden", [NHL, S], F32)    # denominators bounce
    rden = nc.dram_tensor("rden", [NHL, S], F32)    # reciprocals bounce

    with tile.TileContext(nc) as tc:
        with (
            tc.tile_pool(name="qt", bufs=1) as qt_pool,
            tc.tile_pool(name="kt", bufs=1) as kt_pool,
            tc.tile_pool(name="vn", bufs=1) as vn_pool,
            tc.tile_pool(name="cst", bufs=1) as cst,
        ):
            QT = [qt_pool.tile([128, S], BF16, tag=f"qt{d}", name=f"QT{d}")
                  for d in range(DCH)]
            KT = [kt_pool.tile([128, S], BF16, tag=f"kt{d}", name=f"KT{d}")
                  for d in range(DCH)]
            # V natural [seq, 8*(64+1)]: head h cols 65h..65h+63, ones at 65h+64
            VN = [vn_pool.tile([128, NHL * (HD + 1)], BF16, tag=f"vn{i}",
                               name=f"VN{i}")
                  for i in range(NKT)]

            bqs = cst.tile([128, DCH], F32, tag="bqs", name="bqs")
            bks = cst.tile([128, DCH], F32, tag="bks", name="bks")
            bvb = cst.tile([128, DH], F32, tag="bvb", name="bvb")
            nc.sync.dma_start(out=bqs[:, :], in_=bq_c[:, :])
            nc.sync.dma_start(out=bks[:, :], in_=bk_c[:, :])
            nc.sync.dma_start(out=bvb[:, :], in_=bv_b[:, :])

            # causal mask base [128, 896]: mask[x, c] = 1.0 iff c - x >= 384.
            # crossing k-tile i (0..3) of a 512-q tile uses slice
            # mask[:, 384-128i : 896-128i]  ->  valid iff y >= x + 128 i.
            mask = cst.tile([128, 896], BF16, tag="mask", name="mask")
            nc.gpsimd.memset(mask[:, :], 1.0)
            nc.gpsimd.affine_select(
                out=mask[:, :],
                in_=mask[:, :],
                compare_op=mybir.AluOpType.is_ge,
                fill=0.0,
                base=-384,
                pattern=[[1, 896]],
                channel_multiplier=-1,
            )

            ones = cst.tile([128, NHL], F32, tag="ones", name="ones")
            nc.vector.memset(ones[:, :], 1.0)
            for v in VN:
                nc.vector.tensor_copy(v[:, HD::HD + 1], ones[:, :])

            atp_ctx = tc.tile_pool(name="atp", bufs=4)
            atp_pool = atp_ctx.__enter__()
            with (
                tc.tile_pool(name="xv", bufs=1) as xv_pool,
                tc.tile_pool(name="xkq", bufs=1) as xkq_pool,
                tc.tile_pool(name="wp", bufs=1) as wp_pool,
                tc.tile_pool(name="pr", bufs=4) as pr_pool,
                tc.tile_pool(name="nrm", bufs=2) as nrm_pool,
                tc.tile_pool(name="psA", bufs=2, space="PSUM") as psA,
                tc.tile_pool(name="psS", bufs=2, space="PSUM") as psS,
                tc.tile_pool(name="psV", bufs=1, space="PSUM") as psV,
            ):
                wvt = [wp_pool.tile([128, DH], BF16, tag=f"wv{ki}",
                                    name=f"wv{ki}")
                       for ki in range(KCH)]
                wkt = [wp_pool.tile([128, DH], BF16, tag=f"wk{ki}",
                                    name=f"wk{ki}")
                       for ki in range(KCH)]
                wqt = [wp_pool.tile([128, DH], BF16, tag=f"wq{ki}",
                                    name=f"wq{ki}")
                       for ki in range(KCH)]
                xkt = [xkq_pool.tile([128, S], BF16, tag=f"xk{ki}",
                                     name=f"xk{ki}")
                       for ki in range(KCH)]
                xqt = [xkq_pool.tile([128, S], BF16, tag=f"xq{ki}",
                                     name=f"xq{ki}")
                       for ki in range(KCH)]
                for ki in range(KCH):
                    nc.sync.dma_start(out=wvt[ki][:, :],
                                      in_=wv[128 * ki:128 * (ki + 1), :])
                for ki in range(KCH):
                    nc.sync.dma_start(out=wkt[ki][:, :],
                                      in_=wk[128 * ki:128 * (ki + 1), :])
                    nc.sync.dma_start(out=wqt[ki][:, :],
                                      in_=wq[128 * ki:128 * (ki + 1), :])
                for ki in range(KCH):
                    nc.sync.dma_start(out=xkt[ki][:, :],
                                      in_=xk[128 * ki:128 * (ki + 1), :])
                    nc.sync.dma_start(out=xqt[ki][:, :],
                                      in_=xq[128 * ki:128 * (ki + 1), :])

                # ---- V projection (xv streamed in halves on gpsimd queue) --
                for half in range(2):
                    s0 = 1024 * half
                    xvt = [xv_pool.tile([128, 1024], BF16, tag=f"xv{ki}",
                                        name=f"xv{ki}_")
                           for ki in range(KCH)]
                    for ki in range(KCH):
                        nc.gpsimd.dma_start(
                            out=xvt[ki][:, :],
                            in_=xv[128 * ki:128 * (ki + 1), s0:s0 + 1024])
                    for st in range(8):
                        ps = psA.tile([128, DH], F32, tag="psA", name="psAv_")
                        for ki in range(KCH):
                            nc.tensor.matmul(
                                ps[:, :],
                                xvt[ki][:, 128 * st:128 * (st + 1)],
                                wvt[ki][:, :],
                                start=(ki == 0),
                                stop=(ki == KCH - 1),
                            )
                        vdst = VN[8 * half + st][:, :].rearrange(
                            "p (h c) -> p h c", c=HD + 1)[:, :, :HD]
                        nc.vector.tensor_add(
                            vdst,
                            ps[:, :].rearrange("p (h c) -> p h c", c=HD),
                            bvb[:, :].rearrange("p (h c) -> p h c", c=HD),
                        )

                # ---- per pair: K/Q projections then attention ----
                atp_tiles = []
                for p in range(DCH):
                    for wt, xt, dest, bias in ((wkt, xkt, KT, bks),
                                               (wqt, xqt, QT, bqs)):
                        for sc in range(NQT):
                            ps = psA.tile([128, 512], F32, tag="psA",
                                          name="psA_")
                            for ki in range(KCH):
                                nc.tensor.matmul(
                                    ps[:, :],
                                    wt[ki][:, 128 * p:128 * (p + 1)],
                                    xt[ki][:, 512 * sc:512 * (sc + 1)],
                                    start=(ki == 0),
                                    stop=(ki == KCH - 1),
                                )
                            nc.vector.tensor_scalar_add(
                                dest[p][:, 512 * sc:512 * (sc + 1)],
                                ps[:, :],
                                bias[:, p:p + 1],
                            )

                    atp = atp_pool.tile([128, S], BF16, tag="atp",
                                        name="atp_")
                    atp_tiles.append(atp)
                    for j in range(NQT):
                        q0 = 512 * j
                        nk = 4 * (j + 1)  # valid k-tiles (causal)
                        pv = [psV.tile([HD + 1, 512], F32, tag=f"pv{h}",
                                       name=f"pv{h}_")
                              for h in range(2)]
                        for g in range(0, nk, GRP):
                            pss = [psS.tile([128, 512 * GRP], F32, tag="psS",
                                            name="psS_")
                                   for _ in range(2)]
                            for m in range(GRP):
                                k = g + m
                                for h in range(2):
                                    r0 = 64 * h
                                    nc.tensor.matmul(
                                        pss[h][:, 512 * m:512 * (m + 1)],
                                        KT[p][r0:r0 + 64,
                                              128 * k:128 * (k + 1)],
                                        QT[p][r0:r0 + 64, q0:q0 + 512],
                                        start=True,
                                        stop=True,
                                    )
                            prt = [pr_pool.tile([128, 512 * GRP], BF16,
                                                tag="pr", name="pr_")
                                   for _ in range(2)]
                            for h in range(2):
                                nc.scalar.activation(
                                    prt[h][:, :], pss[h][:, :], EXP,
                                    scale=0.125)
                            # causal mask on diagonal-crossing k-tiles
                            for h in range(2):
                                for m in range(GRP):
                                    i = g + m - 4 * j
                                    if 0 <= i <= 3:
                                        msl = mask[:, 384 - 128 * i:
                                                   896 - 128 * i]
                                        nc.vector.tensor_mul(
                                            prt[h][:, 512 * m:512 * (m + 1)],
                                            prt[h][:, 512 * m:512 * (m + 1)],
                                            msl,
                                        )
                            for m in range(GRP):
                                k = g + m
                                for h in range(2):
                                    hl = 2 * p + h
                                    nc.tensor.matmul(
                                        pv[h][:, :],
                                        VN[k][:, 65 * hl:65 * hl + 65],
                                        prt[h][:, 512 * m:512 * (m + 1)],
                                        start=(k == 0),
                                        stop=(k == nk - 1),
                                    )
                        # fast drain: unnormalized numerator -> atp,
                        # denominator row -> DRAM bounce (frees the pv slot)
                        for h in range(2):
                            hl = 2 * p + h
                            nc.vector.tensor_copy(
                                atp[64 * h:64 * h + 64, q0:q0 + 512],
                                pv[h][:HD, :],
                            )
                            drow = nrm_pool.tile([1, 512], F32, tag="drow",
                                                 name="drow_")
                            nc.vector.tensor_copy(drow[:, :],
                                                  pv[h][HD:HD + 1, :])
                            nc.sync.dma_start(
                                out=dden[hl:hl + 1, q0:q0 + 512],
                                in_=drow[:, :],
                            )

                    # per-pair batched normalization, then ship A^T chunk
                    dd = nrm_pool.tile([128, 32], F32, tag="dd", name="dd_")
                    nc.sync.dma_start(
                        out=dd[:, :],
                        in_=dden[2 * p:2 * p + 2, :].rearrange(
                            "h (a f) -> (h a) f", f=32),
                    )
                    rc = nrm_pool.tile([128, 32], F32, tag="rc", name="rc_")
                    nc.vector.reciprocal(rc[:, :], dd[:, :])
                    nc.sync.dma_start(
                        out=rden[2 * p:2 * p + 2, :].rearrange(
                            "h (a f) -> (h a) f", f=32),
                        in_=rc[:, :],
                    )
                    for j in range(NQT):
                        q0 = 512 * j
                        bct = nrm_pool.tile([128, 512], F32, tag="bct",
                                            name="bct_")
                        for h in range(2):
                            nc.sync.dma_start(
                                out=bct[64 * h:64 * h + 64, :],
                                in_=rden[2 * p + h:2 * p + h + 1, q0:q0 + 512]
                                .partition_broadcast(64),
                            )
                        nc.vector.tensor_mul(
                            atp[:, q0:q0 + 512],
                            atp[:, q0:q0 + 512],
                            bct[:, :],
                        )

            # ---------------- Output projection ----------------
            with (
                tc.tile_pool(name="wo", bufs=1) as wo_pool,
                tc.tile_pool(name="ob", bufs=4) as ob_pool,
                tc.tile_pool(name="psO", bufs=4, space="PSUM") as psO,
            ):
                wot = [wo_pool.tile([128, D], BF16, tag=f"wo{c}",
                                    name=f"wot{c}")
                       for c in range(DCH)]
                for c in range(DCH):
                    nc.sync.dma_start(
                        out=wot[c][:, :], in_=wo[128 * c:128 * (c + 1), :])
                for qt in range(NKT):  # 16 q tiles of 128
                    q0 = 128 * qt
                    for n in range(2):
                        ps = psO.tile([128, 512], F32, tag="psO", name="psO_")
                        for c in range(DCH):
                            nc.tensor.matmul(
                                ps[:, :],
                                atp_tiles[c][:, q0:q0 + 128],
                                wot[c][:, 512 * n:512 * (n + 1)],
                                start=(c == 0),
                                stop=(c == DCH - 1),
                            )
                        ot = ob_pool.tile([128, 512], F32, tag="ob",
                                          name="ob_")
                        nc.vector.tensor_copy(ot[:, :], ps[:, :])
                        nc.sync.dma_start(
                            out=out[q0:q0 + 128, 512 * n:512 * (n + 1)],
                            in_=ot[:, :])
            atp_ctx.__exit__(None, None, None)

    nc.compile()
    return nc


def kernel(query, key, value, Wq, bq, Wk, bk, Wv, bv, Wo, bo, **trace_kwargs):
    from concourse.bass_utils import run_bass_kernel_spmd

    global _compiled
    if _compiled is None:
        _compiled = _build()
    nc = _compiled

    import ml_dtypes

    BF = ml_dtypes.bfloat16
    query = np.asarray(query, np.float32)
    key = np.asarray(key, np.float32)
    value = np.asarray(value, np.float32)
    Wq, Wk, Wv, Wo = (np.asarray(w, np.float32) for w in (Wq, Wk, Wv, Wo))
    bq, bk, bv, bo = (np.asarray(b_, np.float32) for b_ in (bq, bk, bv, bo))

    xqT = [np.ascontiguousarray(query[b].T).astype(BF) for b in range(B)]
    xkT = [np.ascontiguousarray(key[b].T).astype(BF) for b in range(B)]
    xvT = [np.ascontiguousarray(value[b].T).astype(BF) for b in range(B)]
    shard = []
    for t in range(TP):
        cs = slice(DH * t, DH * (t + 1))
        shard.append({
            "wq": np.ascontiguousarray(Wq[:, cs]).astype(BF),
            "wk": np.ascontiguousarray(Wk[:, cs]).astype(BF),
            "wv": np.ascontiguousarray(Wv[:, cs]).astype(BF),
            "wo": np.ascontiguousarray(Wo[cs, :]).astype(BF),
            "bq_c": np.ascontiguousarray(bq[cs].reshape(DCH, 128).T),
            "bk_c": np.ascontiguousarray(bk[cs].reshape(DCH, 128).T),
            "bv_b": np.ascontiguousarray(
                np.broadcast_to(bv[cs], (128, DH))),
        })

    in_maps = []
    for c in range(8):
        b, t = c // TP, c % TP
        m = {"xq": xqT[b], "xk": xkT[b], "xv": xvT[b]}
        m.update(shard[t])
        in_maps.append(m)

    res = run_bass_kernel_spmd(nc, in_maps, core_ids=list(range(8)),
                               **trace_kwargs)
    outp = np.empty((B, S, D), np.float32)
    for b in range(B):
        outp[b] = res.results[TP * b]["out"] + res.results[TP * b + 1]["out"] + bo
    if trace_kwargs:
        kernel.last_results = res
    return outp


# revision 20
# speedup vs baseline: 1.1085x; 1.0480x over previous
"""Causal multi-head attention (B=4, S=2048, D=1024, H=16) on 8 TRN2 NeuronCores.

Sharding: DP=4 over batch x TP=2 over heads (8 heads per core). Each core:
  - receives transposed activations xT = x[b].T (host-prepared, bf16),
    column shards of Wq/Wk/Wv (512 cols = 8 heads) and the row shard of Wo.
  - computes V (natural layout, with a ones-column per head that yields the
    softmax denominators inside the PV matmul), then per head-pair p:
    KT[p]/QT[p] projections -> scoresT = K_h Q_h^T (2-head row-packed
    matmuls, causal tile skipping) -> probsT = exp(scoresT/8) * causal mask
    -> PV -> numerator^T + denominator -> batched-reciprocal normalization
    (A^T pair-chunks stay resident in SBUF for the output projection),
    so the ACT-bound attention pipeline overlaps the projection matmuls.
  - finally the partial output A^T.T @ Wo_shard in [seq, D] layout.
  - host sums the two TP partials per batch and adds bo.

All matmul operands are bf16 (1 cycle/column on the PE, half the DMA bytes,
2x DVE modes); accumulation and softmax normalization stay fp32 in PSUM.
All x-activation pools stay live together so no DMA ever waits on compute
for SBUF space (head-of-line blocking on the DMA queue); xv streams on the
gpsimd queue, everything else on sync.
"""

import sys

sys.path.insert(0, "/opt/trn_rl_repo")

import numpy as np

B = 4
S = 2048
D = 1024
H = 16
HD = 64
TP = 2
DH = D // TP          # 512 head-dims per core (8 heads)
NHL = DH // HD        # 8 local heads
DCH = 4               # dchunks of 128 within DH
NKT = S // 128        # 16 key tiles
NQT = S // 512        # 4 query tiles
KCH = D // 128        # 8 contraction tiles for projections
GRP = 2               # score k-tiles grouped per exp op

_compiled = None


def _build():
    import concourse.bacc as bacc
    import concourse.mybir as mybir
    import concourse.tile as tile

    F32 = mybir.dt.float32
    BF16 = mybir.dt.bfloat16
    EXP = mybir.ActivationFunctionType.Exp

    nc = bacc.Bacc("TRN2", target_bir_lowering=False, debug=False)

    xq = nc.dram_tensor("xq", [D, S], BF16, kind="ExternalInput")
    xk = nc.dram_tensor("xk", [D, S], BF16, kind="ExternalInput")
    xv = nc.dram_tensor("xv", [D, S], BF16, kind="ExternalInput")
    wq = nc.dram_tensor("wq", [D, DH], BF16, kind="ExternalInput")
    wk = nc.dram_tensor("wk", [D, DH], BF16, kind="ExternalInput")
    wv = nc.dram_tensor("wv", [D, DH], BF16, kind="ExternalInput")
    wo = nc.dram_tensor("wo", [DH, D], BF16, kind="ExternalInput")
    bq_c = nc.dram_tensor("bq_c", [128, DCH], F32, kind="ExternalInput")
    bk_c = nc.dram_tensor("bk_c", [128, DCH], F32, kind="ExternalInput")
    bv_b = nc.dram_tensor("bv_b", [128, DH], F32, kind="ExternalInput")
    out = nc.dram_tensor("out", [S, D], F32, kind="ExternalOutput")
    dden = nc.dram_tensor("dden", [NHL, S], F32)    # denominators bounce
    rden = nc.dram_tensor("rden", [NHL, S], F32)    # reciprocals bounce

    with tile.TileContext(nc) as tc:
        with (
            tc.tile_pool(name="qt", bufs=1) as qt_pool,
            tc.tile_pool(name="kt", bufs=1) as kt_pool,
            tc.tile_pool(name="vn", bufs=1) as vn_pool,
            tc.tile_pool(name="cst", bufs=1) as cst,
        ):
            QT = [qt_pool.tile([128, S], BF16, tag=f"qt{d}", name=f"QT{d}")
                  for d in range(DCH)]
            KT = [kt_pool.tile([128, S], BF16, tag=f"kt{d}", name=f"KT{d}")
                  for d in range(DCH)]
            # V natural [seq, 8*(64+1)]: head h cols 65h..65h+63, ones at 65h+64
            VN = [vn_pool.tile([128, NHL * (HD + 1)], BF16, tag=f"vn{i}",
                               name=f"VN{i}")
                  for i in range(NKT)]

            bqs = cst.tile([128, DCH], F32, tag="bqs", name="bqs")
            bks = cst.tile([128, DCH], F32, tag="bks", name="bks")
            bvb = cst.tile([128, DH], F32, tag="bvb", name="bvb")
            nc.sync.dma_start(out=bqs[:, :], in_=bq_c[:, :])
            nc.sync.dma_start(out=bks[:, :], in_=bk_c[:, :])
            nc.sync.dma_start(out=bvb[:, :], in_=bv_b[:, :])

            # causal mask base [128, 896]: mask[x, c] = 1.0 iff c - x >= 384.
            # crossing k-tile i (0..3) of a 512-q tile uses slice
            # mask[:, 384-128i : 896-128i]  ->  valid iff y >= x + 128 i.
            mask = cst.tile([128, 896], BF16, tag="mask", name="mask")
            nc.gpsimd.memset(mask[:, :], 1.0)
            nc.gpsimd.affine_select(
                out=mask[:, :],
                in_=mask[:, :],
                compare_op=mybir.AluOpType.is_ge,
                fill=0.0,
                base=-384,
                pattern=[[1, 896]],
                channel_multiplier=-1,
            )

            ones = cst.tile([128, NHL], F32, tag="ones", name="ones")
            nc.vector.memset(ones[:, :], 1.0)
            for v in VN:
                nc.vector.tensor_copy(v[:, HD::HD + 1], ones[:, :])

            atp_ctx = tc.tile_pool(name="atp", bufs=4)
            atp_pool = atp_ctx.__enter__()
            with (
                tc.tile_pool(name="xv", bufs=2) as xv_pool,
                tc.tile_pool(name="xkq", bufs=1) as xkq_pool,
                tc.tile_pool(name="wp", bufs=1) as wp_pool,
                tc.tile_pool(name="pr", bufs=4) as pr_pool,
                tc.tile_pool(name="nrm", bufs=2) as nrm_pool,
                tc.tile_pool(name="psA", bufs=2, space="PSUM") as psA,
                tc.tile_pool(name="psS", bufs=2, space="PSUM") as psS,
                tc.tile_pool(name="psV", bufs=1, space="PSUM") as psV,
            ):
                wvt = [wp_pool.tile([128, DH], BF16, tag=f"wv{ki}",
                                    name=f"wv{ki}")
                       for ki in range(KCH)]
                wkt = [wp_pool.tile([128, DH], BF16, tag=f"wk{ki}",
                                    name=f"wk{ki}")
                       for ki in range(KCH)]
                wqt = [wp_pool.tile([128, DH], BF16, tag=f"wq{ki}",
                                    name=f"wq{ki}")
                       for ki in range(KCH)]
                xkt = [xkq_pool.tile([128, S], BF16, tag=f"xk{ki}",
                                     name=f"xk{ki}")
                       for ki in range(KCH)]
                xqt = [xkq_pool.tile([128, S], BF16, tag=f"xq{ki}",
                                     name=f"xq{ki}")
                       for ki in range(KCH)]
                for ki in range(KCH):
                    nc.sync.dma_start(out=wvt[ki][:, :],
                                      in_=wv[128 * ki:128 * (ki + 1), :])
                xvt_h = [[xv_pool.tile([128, 1024], BF16, tag=f"xv{ki}",
                                       name=f"xv{ki}_{half}")
                          for ki in range(KCH)] for half in range(2)]
                for ki in range(KCH):
                    nc.sync.dma_start(out=xvt_h[0][ki][:, :],
                                      in_=xv[128 * ki:128 * (ki + 1), 0:1024])
                for ki in range(KCH):
                    nc.gpsimd.dma_start(
                        out=xvt_h[1][ki][:, :],
                        in_=xv[128 * ki:128 * (ki + 1), 1024:2048])
                for ki in range(KCH):
                    nc.sync.dma_start(out=wkt[ki][:, :],
                                      in_=wk[128 * ki:128 * (ki + 1), :])
                    nc.sync.dma_start(out=wqt[ki][:, :],
                                      in_=wq[128 * ki:128 * (ki + 1), :])
                for ki in range(KCH):
                    nc.sync.dma_start(out=xkt[ki][:, :],
                                      in_=xk[128 * ki:128 * (ki + 1), :])
                    nc.sync.dma_start(out=xqt[ki][:, :],
                                      in_=xq[128 * ki:128 * (ki + 1), :])

                # ---- V projection ----
                for half in range(2):
                    xvt = xvt_h[half]
                    for st in range(8):
                        ps = psA.tile([128, DH], F32, tag="psA", name="psAv_")
                        for ki in range(KCH):
                            nc.tensor.matmul(
                                ps[:, :],
                                xvt[ki][:, 128 * st:128 * (st + 1)],
                                wvt[ki][:, :],
                                start=(ki == 0),
                                stop=(ki == KCH - 1),
                            )
                        vdst = VN[8 * half + st][:, :].rearrange(
                            "p (h c) -> p h c", c=HD + 1)[:, :, :HD]
                        nc.vector.tensor_add(
                            vdst,
                            ps[:, :].rearrange("p (h c) -> p h c", c=HD),
                            bvb[:, :].rearrange("p (h c) -> p h c", c=HD),
                        )

                # ---- per pair: K/Q projections then attention ----
                atp_tiles = []
                for p in range(DCH):
                    for wt, xt, dest, bias in ((wkt, xkt, KT, bks),
                                               (wqt, xqt, QT, bqs)):
                        for sc in range(NQT):
                            ps = psA.tile([128, 512], F32, tag="psA",
                                          name="psA_")
                            for ki in range(KCH):
                                nc.tensor.matmul(
                                    ps[:, :],
                                    wt[ki][:, 128 * p:128 * (p + 1)],
                                    xt[ki][:, 512 * sc:512 * (sc + 1)],
                                    start=(ki == 0),
                                    stop=(ki == KCH - 1),
                                )
                            nc.vector.tensor_scalar_add(
                                dest[p][:, 512 * sc:512 * (sc + 1)],
                                ps[:, :],
                                bias[:, p:p + 1],
                            )

                    atp = atp_pool.tile([128, S], BF16, tag="atp",
                                        name="atp_")
                    atp_tiles.append(atp)
                    for j in range(NQT):
                        q0 = 512 * j
                        nk = 4 * (j + 1)  # valid k-tiles (causal)
                        pv = [psV.tile([HD + 1, 512], F32, tag=f"pv{h}",
                                       name=f"pv{h}_")
                              for h in range(2)]
                        for g in range(0, nk, GRP):
                            pss = [psS.tile([128, 512 * GRP], F32, tag="psS",
                                            name="psS_")
                                   for _ in range(2)]
                            for m in range(GRP):
                                k = g + m
                                for h in range(2):
                                    r0 = 64 * h
                                    nc.tensor.matmul(
                                        pss[h][:, 512 * m:512 * (m + 1)],
                                        KT[p][r0:r0 + 64,
                                              128 * k:128 * (k + 1)],
                                        QT[p][r0:r0 + 64, q0:q0 + 512],
                                        start=True,
                                        stop=True,
                                    )
                            prt = [pr_pool.tile([128, 512 * GRP], BF16,
                                                tag="pr", name="pr_")
                                   for _ in range(2)]
                            for h in range(2):
                                nc.scalar.activation(
                                    prt[h][:, :], pss[h][:, :], EXP,
                                    scale=0.125)
                            # causal mask on diagonal-crossing k-tiles
                            for h in range(2):
                                for m in range(GRP):
                                    i = g + m - 4 * j
                                    if 0 <= i <= 3:
                                        msl = mask[:, 384 - 128 * i:
                                                   896 - 128 * i]
                                        nc.vector.tensor_mul(
                                            prt[h][:, 512 * m:512 * (m + 1)],
                                            prt[h][:, 512 * m:512 * (m + 1)],
                                            msl,
                                        )
                            for m in range(GRP):
                                k = g + m
                                for h in range(2):
                                    hl = 2 * p + h
                                    nc.tensor.matmul(
                                        pv[h][:, :],
                                        VN[k][:, 65 * hl:65 * hl + 65],
                                        prt[h][:, 512 * m:512 * (m + 1)],
                                        start=(k == 0),
                                        stop=(k == nk - 1),
                                    )
                        # fast drain: unnormalized numerator -> atp,
                        # denominator row -> DRAM bounce (frees the pv slot)
                        for h in range(2):
                            hl = 2 * p + h
                            nc.vector.tensor_copy(
                                atp[64 * h:64 * h + 64, q0:q0 + 512],
                                pv[h][:HD, :],
                            )
                            drow = nrm_pool.tile([1, 512], F32, tag="drow",
                                                 name="drow_")
                            nc.vector.tensor_copy(drow[:, :],
                                                  pv[h][HD:HD + 1, :])
                            nc.sync.dma_start(
                                out=dden[hl:hl + 1, q0:q0 + 512],
                                in_=drow[:, :],
                            )

                    # per-pair batched normalization, then ship A^T chunk
                    dd = nrm_pool.tile([128, 32], F32, tag="dd", name="dd_")
                    nc.sync.dma_start(
                        out=dd[:, :],
                        in_=dden[2 * p:2 * p + 2, :].rearrange(
                            "h (a f) -> (h a) f", f=32),
                    )
                    rc = nrm_pool.tile([128, 32], F32, tag="rc", name="rc_")
                    nc.vector.reciprocal(rc[:, :], dd[:, :])
                    nc.sync.dma_start(
                        out=rden[2 * p:2 * p + 2, :].rearrange(
                            "h (a f) -> (h a) f", f=32),
                        in_=rc[:, :],
                    )
                    for j in range(NQT):
                        q0 = 512 * j
                        bct = nrm_pool.tile([128, 512], F32, tag="bct",
                                            name="bct_")
                        for h in range(2):
                            nc.sync.dma_start(
                                out=bct[64 * h:64 * h + 64, :],
                                in_=rden[2 * p + h:2 * p + h + 1, q0:q0 + 512]
                                .partition_broadcast(64),
                            )
                        nc.vector.tensor_mul(
                            atp[:, q0:q0 + 512],
                            atp[:, q0:q0 + 512],
                            bct[:, :],
                        )

            # ---------------- Output projection ----------------
            with (
                tc.tile_pool(name="wo", bufs=1) as wo_pool,
                tc.tile_pool(name="ob", bufs=4) as ob_pool,
                tc.tile_pool(name="psO", bufs=4, space="PSUM") as psO,
            ):
                wot = [wo_pool.tile([128, D], BF16, tag=f"wo{c}",
                                    name=f"wot{c}")
                       for c in range(DCH)]
                for c in range(DCH):
                    nc.sync.dma_start(
                        out=wot[c][:, :], in_=wo[128 * c:128 * (c + 1), :])
                for qt in range(NKT):  # 16 q tiles of 128
                    q0 = 128 * qt
                    for n in range(2):
                        ps = psO.tile([128, 512], F32, tag="psO", name="psO_")
                        for c in range(DCH):
                            nc.tensor.matmul(
                                ps[:, :],
                                atp_tiles[c][:, q0:q0 + 128],
                                wot[c][:, 512 * n:512 * (n + 1)],
                                start=(c == 0),
                                stop=(c == DCH - 1),
                            )
                        ot = ob_pool.tile([128, 512], F32, tag="ob",
                                          name="ob_")
                        nc.vector.tensor_copy(ot[:, :], ps[:, :])
                        nc.sync.dma_start(
                            out=out[q0:q0 + 128, 512 * n:512 * (n + 1)],
                            in_=ot[:, :])
            atp_ctx.__exit__(None, None, None)

    nc.compile()
    return nc


def kernel(query, key, value, Wq, bq, Wk, bk, Wv, bv, Wo, bo, **trace_kwargs):
    from concourse.bass_utils import run_bass_kernel_spmd

    global _compiled
    if _compiled is None:
        _compiled = _build()
    nc = _compiled

    import ml_dtypes

    BF = ml_dtypes.bfloat16
    query = np.asarray(query, np.float32)
    key = np.asarray(key, np.float32)
    value = np.asarray(value, np.float32)
    Wq, Wk, Wv, Wo = (np.asarray(w, np.float32) for w in (Wq, Wk, Wv, Wo))
    bq, bk, bv, bo = (np.asarray(b_, np.float32) for b_ in (bq, bk, bv, bo))

    xqT = [np.ascontiguousarray(query[b].T).astype(BF) for b in range(B)]
    xkT = [np.ascontiguousarray(key[b].T).astype(BF) for b in range(B)]
    xvT = [np.ascontiguousarray(value[b].T).astype(BF) for b in range(B)]
    shard = []
    for t in range(TP):
        cs = slice(DH * t, DH * (t + 1))
        shard.append({
            "wq": np.ascontiguousarray(Wq[:, cs]).astype(BF),
            "wk": np.ascontiguousarray(Wk[:, cs]).astype(BF),
            "wv": np.ascontiguousarray(Wv[:, cs]).astype(BF),
            "wo": np.ascontiguousarray(Wo[cs, :]).astype(BF),
            "bq_c": np.ascontiguousarray(bq[cs].reshape(DCH, 128).T),
            "bk_c": np.ascontiguousarray(bk[cs].reshape(DCH, 128).T),
            "bv_b": np.ascontiguousarray(
                np.broadcast_to(bv[cs], (128, DH))),
        })

    in_maps = []
    for c in range(8):
        b, t = c // TP, c % TP
        m = {"xq": xqT[b], "xk": xkT[b], "xv": xvT[b]}
        m.update(shard[t])
        in_maps.append(m)

    res = run_bass_kernel_spmd(nc, in_maps, core_ids=list(range(8)),
                               **trace_kwargs)
    outp = np.empty((B, S, D), np.float32)
    for b in range(B):
        outp[b] = res.results[TP * b]["out"] + res.results[TP * b + 1]["out"] + bo
    if trace_kwargs:
        kernel.last_results = res
    return outp
